# revision 9
# baseline (speedup 1.0000x reference)
"""Trainium2 Bass kernel for CalcSpixelFeats (superpixel feature aggregation).

Strategy ("sorted-segment matmul"):
  - 8 NeuronCores, each handles half an image (4 images x 2 pixel-halves).
  - Host-side sharding/layout: stable-sort each core's pixels by their base
    superpixel id. Every 128-pixel tile then touches at most 3 id-groups
    (group sizes ~ 128 +- 11), split at two boundary offsets s1 <= s2.
  - Device: per tile ONE matmul out[27, 33] = lhsT[128, 27]^T @ F_ext[128, 33]
    where lhsT = [W9*maskA | W9*mask(A u B) | W9] (masks built by fused DVE
    compare-multiply from the boundary offsets), and F_ext = 32 channels + a
    ones column (gives the weight sums). Blocks accumulate in rotating PSUM
    banks, flushed to SBUF by the scalar engine, one output DMA at the end.
  - Host-side unshard: prefix-difference the 3 planes into per-group partial
    sums, scatter-add into per-image bins, apply the 3x3 neighbor offsets
    with validity, divide by weight sums.

All arithmetic on-device is fp32; no approximation anywhere.
"""
import sys

sys.path.insert(0, "/opt/trn_rl_repo")

import numpy as np

B, C, H, W = 4, 32, 256, 256
NW = NH = 16
K = NW * NH
PIX = H * W // 2          # pixels per core (half image)
TILE = 128
T = PIX // TILE           # 256 tiles per core
NCOLS = C + 1             # 33: channels + ones column
MCOLS = 27                # 3 planes x 9
BPB = 15                  # [27, 33] blocks per PSUM bank (495 of 512 fp32)
NFLUSH = (T + BPB - 1) // BPB
STG = NFLUSH * BPB * NCOLS  # 8910 staging columns
CHUNK = 64                # tiles per DMA chunk
N_CORES = 8

_compiled = None


class _CompiledKernel:
    """Compile a finalized Bass module once; run SPMD on 8 cores via PJRT."""

    def __init__(self, nc, n_cores):
        import jax
        import concourse.mybir as mybir
        from concourse.bass2jax import (
            _bass_exec_p, partition_id_tensor, install_neuronx_cc_hook)
        from jax.sharding import Mesh, PartitionSpec
        from jax.experimental.shard_map import shard_map

        install_neuronx_cc_hook()
        if not nc.is_finalized():
            nc.finalize()
        self.n_cores = n_cores
        self._jax = jax
        partition_name = (nc.partition_id_tensor.name
                          if nc.partition_id_tensor else None)
        in_names, out_names, out_avals = [], [], []
        for alloc in nc.m.functions[0].allocations:
            if not isinstance(alloc, mybir.MemoryLocationSet):
                continue
            name = alloc.memorylocations[0].name
            if alloc.kind == "ExternalInput":
                if name != partition_name:
                    in_names.append(name)
            elif alloc.kind == "ExternalOutput":
                out_names.append(name)
                out_avals.append(jax.core.ShapedArray(
                    tuple(alloc.tensor_shape), mybir.dt.np(alloc.dtype)))
        self.in_names, self.out_names, self.out_avals = in_names, out_names, out_avals
        n_params, n_outs = len(in_names), len(out_avals)
        all_in_names = in_names + out_names
        if partition_name is not None:
            all_in_names.append(partition_name)

        def _body(*args):
            operands = list(args)
            if partition_name is not None:
                operands.append(partition_id_tensor())
            return tuple(_bass_exec_p.bind(
                *operands,
                out_avals=tuple(out_avals),
                in_names=tuple(all_in_names),
                out_names=tuple(out_names),
                lowering_input_output_aliases=(),
                sim_require_finite=True,
                sim_require_nnan=True,
                nc=nc,
            ))

        devices = jax.devices()[:n_cores]
        mesh = Mesh(np.asarray(devices), ("core",))
        self.fn = jax.jit(
            shard_map(_body, mesh=mesh,
                      in_specs=(PartitionSpec("core"),) * (n_params + n_outs),
                      out_specs=(PartitionSpec("core"),) * n_outs,
                      check_rep=False),
            keep_unused=True,
        )
        self._dev_args = None

    def set_inputs(self, in_maps):
        jax = self._jax
        concat_in = [
            np.concatenate([np.asarray(in_maps[c][name])
                            for c in range(self.n_cores)], axis=0)
            for name in self.in_names
        ]
        concat_zeros = [
            np.zeros((self.n_cores * a.shape[0], *a.shape[1:]), a.dtype)
            for a in self.out_avals
        ]
        self._dev_args = ([jax.device_put(a) for a in concat_in]
                          + [jax.device_put(z) for z in concat_zeros])

    def run_blocking(self):
        outs = self.fn(*self._dev_args)
        self._jax.block_until_ready(outs)
        return outs

    def get_results(self):
        outs = self.run_blocking()
        res = []
        for c in range(self.n_cores):
            d = {}
            for i, name in enumerate(self.out_names):
                per = np.asarray(outs[i]).reshape(
                    self.n_cores, *self.out_avals[i].shape)
                d[name] = per[c]
            res.append(d)
        return res


def _build_device():
    import concourse.bacc as bacc
    import concourse.mybir as mybir
    from concourse import tile
    CompiledKernel = _CompiledKernel

    DT = mybir.dt.float32
    nc = bacc.Bacc("TRN2", target_bir_lowering=False, debug=False,
                   num_devices=N_CORES)
    Fs = nc.dram_tensor("Fs", [TILE, T * NCOLS], DT, kind="ExternalInput")
    Ws = nc.dram_tensor("Ws", [TILE, T * 9], DT, kind="ExternalInput")
    SA = nc.dram_tensor("SPLA", [TILE, T], DT, kind="ExternalInput")
    SB = nc.dram_tensor("SPLB", [TILE, T], DT, kind="ExternalInput")
    PIDX = nc.dram_tensor("PIDX", [TILE, 1], DT, kind="ExternalInput")
    OUTS = nc.dram_tensor("OUTS", [MCOLS, STG], DT, kind="ExternalOutput")

    nchunks = T // CHUNK
    with tile.TileContext(nc) as tc:
        with (
            tc.tile_pool(name="fp", bufs=3) as fp,
            tc.tile_pool(name="cp", bufs=1) as cp,
            tc.tile_pool(name="ps", bufs=1, space="PSUM") as ps,
        ):
            wp = fp
            sp = fp
            pidx_t = cp.tile([TILE, 1], DT)
            nc.sync.dma_start(out=pidx_t[:], in_=PIDX[:])
            stage_t = cp.tile([MCOLS, STG], DT)
            psums = [ps.tile([MCOLS, BPB * NCOLS], mybir.dt.float32,
                             name=f"psbank{i}") for i in range(8)]
            for k in range(nchunks):
                f_t = fp.tile([TILE, CHUNK, NCOLS], DT)
                nc.sync.dma_start(
                    out=f_t[:],
                    in_=Fs[:, k * CHUNK * NCOLS:(k + 1) * CHUNK * NCOLS])
                ws_t = wp.tile([TILE, CHUNK, 9], DT, name="ws_t")
                wcat = wp.tile([TILE, CHUNK, MCOLS], DT, name="wcat")
                nc.sync.dma_start(
                    out=ws_t[:],
                    in_=Ws[:, k * CHUNK * 9:(k + 1) * CHUNK * 9])
                sa_t = sp.tile([TILE, CHUNK], DT, name="sa_t")
                sb_t = sp.tile([TILE, CHUNK], DT, name="sb_t")
                nc.sync.dma_start(out=sa_t[:], in_=SA[:, k * CHUNK:(k + 1) * CHUNK])
                nc.sync.dma_start(out=sb_t[:], in_=SB[:, k * CHUNK:(k + 1) * CHUNK])
                # cols 0:9 = W * (p < s1); 9:18 = W * (p < s2); 18:27 = W
                nc.vector.scalar_tensor_tensor(
                    wcat[:, :, 0:9],
                    sa_t[:].unsqueeze(-1).broadcast_to([TILE, CHUNK, 9]),
                    pidx_t[:],
                    ws_t[:],
                    op0=mybir.AluOpType.is_gt,
                    op1=mybir.AluOpType.mult,
                )
                nc.vector.scalar_tensor_tensor(
                    wcat[:, :, 9:18],
                    sb_t[:].unsqueeze(-1).broadcast_to([TILE, CHUNK, 9]),
                    pidx_t[:],
                    ws_t[:],
                    op0=mybir.AluOpType.is_gt,
                    op1=mybir.AluOpType.mult,
                )
                nc.vector.tensor_copy(wcat[:, :, 18:27], ws_t[:])
                for tt in range(CHUNK):
                    t = k * CHUNK + tt
                    bank = (t // BPB) % 8
                    slot = t % BPB
                    nc.tensor.matmul(
                        psums[bank][:, slot * NCOLS:(slot + 1) * NCOLS],
                        wcat[:, tt, :],
                        f_t[:, tt, :],
                        start=True, stop=True, skip_group_check=True,
                    )
                    if slot == BPB - 1 or t == T - 1:
                        fl = t // BPB
                        nc.scalar.copy(
                            out=stage_t[:, fl * BPB * NCOLS:(fl + 1) * BPB * NCOLS],
                            in_=psums[bank][:],
                        )
            nc.sync.dma_start(out=OUTS[:], in_=stage_t[:])
    return CompiledKernel(nc, N_CORES)


def _get_compiled():
    global _compiled
    if _compiled is None:
        _compiled = _build_device()
    return _compiled


def _prep_core(pf_half, am_half, idx_half):
    """pf_half: [C, PIX] f32, am_half: [9, PIX] f32, idx_half: [PIX] int.
    Returns (device input dict, (gA, gB, gC) merge metadata)."""
    order = np.argsort(idx_half, kind="stable")
    sid = idx_half[order].reshape(T, TILE)
    gA = sid[:, 0]
    neq = sid != gA[:, None]
    s1 = np.where(neq.any(1), neq.argmax(1), TILE).astype(np.int64)
    gB = sid[np.arange(T), np.minimum(s1, TILE - 1)]
    neq2 = (sid != gB[:, None]) & (np.arange(TILE)[None, :] >= s1[:, None])
    s2 = np.where(neq2.any(1), neq2.argmax(1), TILE).astype(np.int64)
    gC = sid[np.arange(T), np.minimum(s2, TILE - 1)]
    if (s2 < TILE).any():
        bad = np.nonzero(s2 < TILE)[0]
        for t in bad:
            assert (sid[t, s2[t]:] == gC[t]).all(), "tile spans >3 groups"

    Fs = np.empty((TILE, T, NCOLS), np.float32)
    Fs[:, :, :C] = pf_half[:, order].reshape(C, T, TILE).transpose(2, 1, 0)
    Fs[:, :, C] = 1.0
    Wso = am_half[:, order].reshape(9, T, TILE).transpose(2, 1, 0)
    inp = dict(
        Fs=np.ascontiguousarray(Fs.reshape(TILE, T * NCOLS)),
        Ws=np.ascontiguousarray(Wso.reshape(TILE, T * 9)),
        SPLA=np.ascontiguousarray(
            np.broadcast_to(s1.astype(np.float32), (TILE, T))),
        SPLB=np.ascontiguousarray(
            np.broadcast_to(s2.astype(np.float32), (TILE, T))),
        PIDX=np.arange(TILE, dtype=np.float32).reshape(TILE, 1),
    )
    return inp, (gA.astype(np.int64), gB.astype(np.int64), gC.astype(np.int64))


def _merge_core(outs, meta, bins):
    gA, gB, gC = meta
    blocks = outs.reshape(MCOLS, NFLUSH * BPB, NCOLS).transpose(1, 0, 2)[:T]
    cA = blocks[:, 0:9, :]
    cB = blocks[:, 9:18, :] - cA
    cC = blocks[:, 18:27, :] - blocks[:, 9:18, :]
    np.add.at(bins, gA, cA)
    np.add.at(bins, gB, cB)
    np.add.at(bins, gC, cC)


def _finalize(bins_all):
    ks = np.arange(K)
    ix, iy = ks % NW, ks // NW
    fsum = np.zeros((B, C, K), np.float64)
    wsum = np.zeros((B, K), np.float64)
    j = 0
    for dy in (-1, 0, 1):
        for dx in (-1, 0, 1):
            tx, ty = ix + dx, iy + dy
            valid = (tx >= 0) & (tx < NW) & (ty >= 0) & (ty < NH)
            tgt = (ty * NW + tx)[valid]
            src = ks[valid]
            for b in range(B):
                np.add.at(fsum[b].T, tgt, bins_all[b, src, j, :C])
                np.add.at(wsum[b], tgt, bins_all[b, src, j, C])
            j += 1
    eps = 1e-16
    denom = np.where(wsum > eps, wsum, 1.0)
    out = np.where(wsum[:, None, :] > eps, fsum / denom[:, None, :], 0.0)
    return out.astype(np.float32)


def kernel(pixel_feats, assoc_map, index_map, nw_spixels, nh_spixels):
    assert int(nw_spixels) == NW and int(nh_spixels) == NH
    pixel_feats = np.asarray(pixel_feats, dtype=np.float32)
    assoc_map = np.asarray(assoc_map, dtype=np.float32)
    index_map = np.asarray(index_map)

    in_maps, metas = [], []
    for b in range(B):
        pf = pixel_feats[b].reshape(C, 2, PIX)
        am = assoc_map[b].reshape(9, 2, PIX)
        im = index_map[b].reshape(2, PIX)
        for h in range(2):
            inp, meta = _prep_core(pf[:, h], am[:, h], im[h].astype(np.int64))
            in_maps.append(inp)
            metas.append(meta)

    ck = _get_compiled()
    ck.set_inputs(in_maps)
    results = ck.get_results()

    bins_all = np.zeros((B, K, 9, NCOLS), np.float32)
    for core in range(N_CORES):
        _merge_core(results[core]["OUTS"], metas[core], bins_all[core // 2])
    return _finalize(bins_all)


# revision 10
# speedup vs baseline: 1.3418x; 1.3418x over previous
"""Trainium2 Bass kernel for CalcSpixelFeats (superpixel feature aggregation).

Strategy ("sorted-segment matmul"):
  - 8 NeuronCores, each handles half an image (4 images x 2 pixel-halves).
  - Host-side sharding/layout: stable-sort each core's pixels by their base
    superpixel id. Every 128-pixel tile then touches at most 3 id-groups
    (group sizes ~ 128 +- 11), split at two boundary offsets s1 <= s2.
  - Device: per tile ONE matmul out[27, 33] = lhsT[128, 27]^T @ F_ext[128, 33]
    where lhsT = [W9*maskA | W9*mask(A u B) | W9] (masks built by fused DVE
    compare-multiply from the boundary offsets), and F_ext = 32 channels + a
    ones column (gives the weight sums). Blocks accumulate in rotating PSUM
    banks, flushed to SBUF by the scalar engine, one output DMA at the end.
  - Host-side unshard: prefix-difference the 3 planes into per-group partial
    sums, scatter-add into per-image bins, apply the 3x3 neighbor offsets
    with validity, divide by weight sums.

All arithmetic on-device is fp32; no approximation anywhere.
"""
import sys

sys.path.insert(0, "/opt/trn_rl_repo")

import numpy as np

B, C, H, W = 4, 32, 256, 256
NW = NH = 16
K = NW * NH
PIX = H * W // 2          # pixels per core (half image)
TILE = 128
T = PIX // TILE           # 256 tiles per core
NCOLS = C + 1             # 33: channels + ones column
MCOLS = 27                # 3 planes x 9
BPB = 15                  # 33-col block slots per PSUM bank (495 of 512 fp32)
NBANKS = 5                # col-packed: 4 groups x 15 slots = 60 tiles per bank
STG = NBANKS * BPB * NCOLS  # 2475 staging columns (x128 partitions)
CHUNK = 64                # tiles per DMA chunk
N_CORES = 8

_compiled = None


class _CompiledKernel:
    """Compile a finalized Bass module once; run SPMD on 8 cores via PJRT."""

    def __init__(self, nc, n_cores):
        import jax
        import concourse.mybir as mybir
        from concourse.bass2jax import (
            _bass_exec_p, partition_id_tensor, install_neuronx_cc_hook)
        from jax.sharding import Mesh, PartitionSpec
        from jax.experimental.shard_map import shard_map

        install_neuronx_cc_hook()
        if not nc.is_finalized():
            nc.finalize()
        self.n_cores = n_cores
        self._jax = jax
        partition_name = (nc.partition_id_tensor.name
                          if nc.partition_id_tensor else None)
        in_names, out_names, out_avals = [], [], []
        for alloc in nc.m.functions[0].allocations:
            if not isinstance(alloc, mybir.MemoryLocationSet):
                continue
            name = alloc.memorylocations[0].name
            if alloc.kind == "ExternalInput":
                if name != partition_name:
                    in_names.append(name)
            elif alloc.kind == "ExternalOutput":
                out_names.append(name)
                out_avals.append(jax.core.ShapedArray(
                    tuple(alloc.tensor_shape), mybir.dt.np(alloc.dtype)))
        self.in_names, self.out_names, self.out_avals = in_names, out_names, out_avals
        n_params, n_outs = len(in_names), len(out_avals)
        all_in_names = in_names + out_names
        if partition_name is not None:
            all_in_names.append(partition_name)

        def _body(*args):
            operands = list(args)
            if partition_name is not None:
                operands.append(partition_id_tensor())
            return tuple(_bass_exec_p.bind(
                *operands,
                out_avals=tuple(out_avals),
                in_names=tuple(all_in_names),
                out_names=tuple(out_names),
                lowering_input_output_aliases=(),
                sim_require_finite=True,
                sim_require_nnan=True,
                nc=nc,
            ))

        devices = jax.devices()[:n_cores]
        mesh = Mesh(np.asarray(devices), ("core",))
        self.fn = jax.jit(
            shard_map(_body, mesh=mesh,
                      in_specs=(PartitionSpec("core"),) * (n_params + n_outs),
                      out_specs=(PartitionSpec("core"),) * n_outs,
                      check_rep=False),
            keep_unused=True,
        )
        self._dev_args = None

    def set_inputs(self, in_maps):
        jax = self._jax
        concat_in = [
            np.concatenate([np.asarray(in_maps[c][name])
                            for c in range(self.n_cores)], axis=0)
            for name in self.in_names
        ]
        concat_zeros = [
            np.zeros((self.n_cores * a.shape[0], *a.shape[1:]), a.dtype)
            for a in self.out_avals
        ]
        self._dev_args = ([jax.device_put(a) for a in concat_in]
                          + [jax.device_put(z) for z in concat_zeros])

    def run_blocking(self):
        outs = self.fn(*self._dev_args)
        self._jax.block_until_ready(outs)
        return outs

    def get_results(self):
        outs = self.run_blocking()
        res = []
        for c in range(self.n_cores):
            d = {}
            for i, name in enumerate(self.out_names):
                per = np.asarray(outs[i]).reshape(
                    self.n_cores, *self.out_avals[i].shape)
                d[name] = per[c]
            res.append(d)
        return res


def _build_device():
    import concourse.bacc as bacc
    import concourse.mybir as mybir
    from concourse import tile
    CompiledKernel = _CompiledKernel

    DT = mybir.dt.float32
    nc = bacc.Bacc("TRN2", target_bir_lowering=False, debug=False,
                   num_devices=N_CORES)
    Fs = nc.dram_tensor("Fs", [TILE, T * NCOLS], DT, kind="ExternalInput")
    Ws = nc.dram_tensor("Ws", [TILE, T * 9], DT, kind="ExternalInput")
    SA = nc.dram_tensor("SPLA", [TILE, T], DT, kind="ExternalInput")
    SB = nc.dram_tensor("SPLB", [TILE, T], DT, kind="ExternalInput")
    PIDX = nc.dram_tensor("PIDX", [TILE, 1], DT, kind="ExternalInput")
    OUTS = nc.dram_tensor("OUTS", [128, STG], DT, kind="ExternalOutput")

    nchunks = T // CHUNK
    with tile.TileContext(nc) as tc:
        with (
            tc.tile_pool(name="fp", bufs=3) as fp,
            tc.tile_pool(name="cp", bufs=1) as cp,
            tc.tile_pool(name="ps", bufs=1, space="PSUM") as ps,
        ):
            wp = fp
            sp = fp
            pidx_t = cp.tile([TILE, 1], DT)
            nc.sync.dma_start(out=pidx_t[:], in_=PIDX[:])
            stage_t = cp.tile([128, STG], DT)
            psums = [ps.tile([128, BPB * NCOLS], mybir.dt.float32,
                             name=f"psbank{i}") for i in range(NBANKS)]
            for k in range(nchunks):
                f_t = fp.tile([TILE, CHUNK, NCOLS], DT)
                nc.sync.dma_start(
                    out=f_t[:],
                    in_=Fs[:, k * CHUNK * NCOLS:(k + 1) * CHUNK * NCOLS])
                ws_t = wp.tile([TILE, CHUNK, 9], DT, name="ws_t")
                wcat = wp.tile([TILE, CHUNK, MCOLS], DT, name="wcat")
                nc.sync.dma_start(
                    out=ws_t[:],
                    in_=Ws[:, k * CHUNK * 9:(k + 1) * CHUNK * 9])
                sa_t = sp.tile([TILE, CHUNK], DT, name="sa_t")
                sb_t = sp.tile([TILE, CHUNK], DT, name="sb_t")
                nc.sync.dma_start(out=sa_t[:], in_=SA[:, k * CHUNK:(k + 1) * CHUNK])
                nc.sync.dma_start(out=sb_t[:], in_=SB[:, k * CHUNK:(k + 1) * CHUNK])
                # cols 0:9 = W * (p < s1); 9:18 = W * (p < s2); 18:27 = W
                nc.vector.scalar_tensor_tensor(
                    wcat[:, :, 0:9],
                    sa_t[:].unsqueeze(-1).broadcast_to([TILE, CHUNK, 9]),
                    pidx_t[:],
                    ws_t[:],
                    op0=mybir.AluOpType.is_gt,
                    op1=mybir.AluOpType.mult,
                )
                nc.vector.scalar_tensor_tensor(
                    wcat[:, :, 9:18],
                    sb_t[:].unsqueeze(-1).broadcast_to([TILE, CHUNK, 9]),
                    pidx_t[:],
                    ws_t[:],
                    op0=mybir.AluOpType.is_gt,
                    op1=mybir.AluOpType.mult,
                )
                nc.vector.tensor_copy(wcat[:, :, 18:27], ws_t[:])
                for tt in range(CHUNK):
                    t = k * CHUNK + tt
                    bank = t // 60
                    idx = t % 60
                    g = idx % 4
                    slot = idx // 4
                    nc.tensor.matmul(
                        psums[bank][32 * g:32 * g + MCOLS,
                                    slot * NCOLS:(slot + 1) * NCOLS],
                        wcat[:, tt, :],
                        f_t[:, tt, :],
                        start=True, stop=True, skip_group_check=True,
                        tile_position=(0, 32 * g),
                    )
                    if t in (59, 119, 179, 239, T - 1):
                        nc.scalar.copy(
                            out=stage_t[:, bank * BPB * NCOLS:(bank + 1) * BPB * NCOLS],
                            in_=psums[bank][:],
                        )
            nc.sync.dma_start(out=OUTS[:], in_=stage_t[:])
    return CompiledKernel(nc, N_CORES)


def _get_compiled():
    global _compiled
    if _compiled is None:
        _compiled = _build_device()
    return _compiled


def _prep_core(pf_half, am_half, idx_half):
    """pf_half: [C, PIX] f32, am_half: [9, PIX] f32, idx_half: [PIX] int.
    Returns (device input dict, (gA, gB, gC) merge metadata)."""
    order = np.argsort(idx_half, kind="stable")
    sid = idx_half[order].reshape(T, TILE)
    gA = sid[:, 0]
    neq = sid != gA[:, None]
    s1 = np.where(neq.any(1), neq.argmax(1), TILE).astype(np.int64)
    gB = sid[np.arange(T), np.minimum(s1, TILE - 1)]
    neq2 = (sid != gB[:, None]) & (np.arange(TILE)[None, :] >= s1[:, None])
    s2 = np.where(neq2.any(1), neq2.argmax(1), TILE).astype(np.int64)
    gC = sid[np.arange(T), np.minimum(s2, TILE - 1)]
    if (s2 < TILE).any():
        bad = np.nonzero(s2 < TILE)[0]
        for t in bad:
            assert (sid[t, s2[t]:] == gC[t]).all(), "tile spans >3 groups"

    Fs = np.empty((TILE, T, NCOLS), np.float32)
    Fs[:, :, :C] = pf_half[:, order].reshape(C, T, TILE).transpose(2, 1, 0)
    Fs[:, :, C] = 1.0
    Wso = am_half[:, order].reshape(9, T, TILE).transpose(2, 1, 0)
    inp = dict(
        Fs=np.ascontiguousarray(Fs.reshape(TILE, T * NCOLS)),
        Ws=np.ascontiguousarray(Wso.reshape(TILE, T * 9)),
        SPLA=np.ascontiguousarray(
            np.broadcast_to(s1.astype(np.float32), (TILE, T))),
        SPLB=np.ascontiguousarray(
            np.broadcast_to(s2.astype(np.float32), (TILE, T))),
        PIDX=np.arange(TILE, dtype=np.float32).reshape(TILE, 1),
    )
    return inp, (gA.astype(np.int64), gB.astype(np.int64), gC.astype(np.int64))


def _merge_core(outs, meta, bins):
    gA, gB, gC = meta
    outs_r = outs.reshape(128, NBANKS, BPB, NCOLS)
    ts = np.arange(T)
    bk, idx = ts // 60, ts % 60
    g, sl = idx % 4, idx // 4
    rows = 32 * g[:, None] + np.arange(MCOLS)[None, :]
    blocks = outs_r[rows, bk[:, None], sl[:, None], :]
    cA = blocks[:, 0:9, :]
    cB = blocks[:, 9:18, :] - cA
    cC = blocks[:, 18:27, :] - blocks[:, 9:18, :]
    np.add.at(bins, gA, cA)
    np.add.at(bins, gB, cB)
    np.add.at(bins, gC, cC)


def _finalize(bins_all):
    ks = np.arange(K)
    ix, iy = ks % NW, ks // NW
    fsum = np.zeros((B, C, K), np.float64)
    wsum = np.zeros((B, K), np.float64)
    j = 0
    for dy in (-1, 0, 1):
        for dx in (-1, 0, 1):
            tx, ty = ix + dx, iy + dy
            valid = (tx >= 0) & (tx < NW) & (ty >= 0) & (ty < NH)
            tgt = (ty * NW + tx)[valid]
            src = ks[valid]
            for b in range(B):
                np.add.at(fsum[b].T, tgt, bins_all[b, src, j, :C])
                np.add.at(wsum[b], tgt, bins_all[b, src, j, C])
            j += 1
    eps = 1e-16
    denom = np.where(wsum > eps, wsum, 1.0)
    out = np.where(wsum[:, None, :] > eps, fsum / denom[:, None, :], 0.0)
    return out.astype(np.float32)


def kernel(pixel_feats, assoc_map, index_map, nw_spixels, nh_spixels):
    assert int(nw_spixels) == NW and int(nh_spixels) == NH
    pixel_feats = np.asarray(pixel_feats, dtype=np.float32)
    assoc_map = np.asarray(assoc_map, dtype=np.float32)
    index_map = np.asarray(index_map)

    in_maps, metas = [], []
    for b in range(B):
        pf = pixel_feats[b].reshape(C, 2, PIX)
        am = assoc_map[b].reshape(9, 2, PIX)
        im = index_map[b].reshape(2, PIX)
        for h in range(2):
            inp, meta = _prep_core(pf[:, h], am[:, h], im[h].astype(np.int64))
            in_maps.append(inp)
            metas.append(meta)

    ck = _get_compiled()
    ck.set_inputs(in_maps)
    results = ck.get_results()

    bins_all = np.zeros((B, K, 9, NCOLS), np.float32)
    for core in range(N_CORES):
        _merge_core(results[core]["OUTS"], metas[core], bins_all[core // 2])
    return _finalize(bins_all)
